# revision 42
# baseline (speedup 1.0000x reference)
"""Trainium2 Bass kernel for nn_Attention (qkv+BN -> biased softmax attention -> gelu -> proj+BN).

Sharding: data-parallel over batch B=128 across 8 NeuronCores (16 batches each).
BatchNorm (training-mode) statistics are all-reduced across cores (tiny collectives).

v2 layout strategy (vs v1): keep the PE tensor engine continuously busy so the
TRN2 p-state ramp holds at full clock, and strip non-essential PE work.
  - Phase A: Gram-matrix BN1 stats computed FIRST; the stats AllReduce overlaps
    the q,k projection matmuls and the first two batches' v-production.
  - q,k BN scaling applied lazily per batch (spread across the loop).
  - softmax divide: reciprocal row-sums broadcast across partitions on GpSimd
    (partition_broadcast) instead of PE ones-outer matmuls; the divide itself is
    fused into the AV PSUM->SBUF copy via scalar_tensor_tensor.
  - attention outputs collect into per-4-batch av_group tiles [128, 4*8*320];
    gelu runs once per head per group (strided AP), so Exp<->Gelu activation
    table switches amortize over 4 batches.
  - output projection (proj) chains for group G are drained as PE filler during
    group G+1's batches; BN2 sums/sumsq accumulate per chunk via accum_out.
  - v row-major production for batch b+2 is interleaved between AV chains as PE
    filler (also hides PSUM bank reuse latency).
"""
import os
from contextlib import ExitStack

import numpy as np
import ml_dtypes

import concourse.bass as bass
import concourse.tile as tile
from concourse import bacc, mybir
from concourse.bass_utils import run_bass_kernel_spmd

NCORES = int(os.environ.get("KERN_NCORES", "8"))
TRACE_SIM = os.environ.get("KERN_TRACE_SIM", "") == "1"
DBG = os.environ.get("KERN_DBG", "") == "1"
SIM_SAFE = os.environ.get("KERN_SIM_SAFE", "") == "1"
B, N, C = 128, 320, 256
NH, DK, DV = 8, 32, 128
H = NH * (2 * DK + DV)       # 1536
DH = NH * DV                 # 1024
BL = B // 8                  # 16 batches/core (fixed shard size)
R = BL * N                   # 5120 rows/core
NT = B * N                   # 40960 global rows
EPS = 1e-5
SCALE = DK ** -0.5
FP = mybir.dt.float32
BF = mybir.dt.bfloat16

NHC = H // 128               # 12 h-chunks
NRB = R // 512               # 10 row blocks of 512
MCS = [128, 128, 64]         # chunking of N=320
AF = mybir.ActivationFunctionType
OP = mybir.AluOpType

# gelu/proj group boundaries (batch index ranges)
GROUPS = [(2 * i, 2) for i in range(7)] + [(14, 1), (15, 1)]


def build_program():
    nc = bacc.Bacc("TRN2", target_bir_lowering=False, debug=False,
                   enable_asserts=False, num_devices=NCORES)
    xT_d = nc.dram_tensor("xT", [C, R], BF, kind="ExternalInput").ap()
    xb_d = nc.dram_tensor("xb", [R, C + 1], BF, kind="ExternalInput").ap()
    wqkvT_d = nc.dram_tensor("wqkvT", [C, H], BF, kind="ExternalInput").ap()
    wprojT_d = nc.dram_tensor("wprojT", [DH, C], BF, kind="ExternalInput").ap()
    eb4_d = nc.dram_tensor("eb4", [2, 3, 128, 4 * N], BF, kind="ExternalInput").ap()
    g1_d = nc.dram_tensor("g1c", [128, NHC], FP, kind="ExternalInput").ap()
    b1_d = nc.dram_tensor("b1c", [128, NHC], FP, kind="ExternalInput").ap()
    g2_d = nc.dram_tensor("g2c", [128, 2], FP, kind="ExternalInput").ap()
    b2_d = nc.dram_tensor("b2c", [128, 2], FP, kind="ExternalInput").ap()
    yT_d = nc.dram_tensor("yT", [C, R], BF, kind="ExternalOutput").ap()
    dbg_d = nc.dram_tensor("dbg", [128, 5120], BF,
                           kind="ExternalOutput").ap() if DBG else None
    dbg2_d = nc.dram_tensor("dbg2", [128, 48], FP,
                            kind="ExternalOutput").ap() if DBG else None

    es = ExitStack()
    with es:
        tc = es.enter_context(tile.TileContext(nc, trace_sim=TRACE_SIM))
        constp = es.enter_context(tc.tile_pool(name="const", bufs=1))
        qkvp = es.enter_context(tc.tile_pool(name="qkv", bufs=1))
        dramp = es.enter_context(tc.tile_pool(name="dram", bufs=1, space="DRAM"))
        statp = es.enter_context(tc.tile_pool(name="stat", bufs=1))
        xp = es.enter_context(tc.tile_pool(name="xa", bufs=1))
        pvp = es.enter_context(tc.tile_pool(name="pv", bufs=1, space="PSUM"))
        vbp = es.enter_context(tc.tile_pool(name="vb", bufs=18))

        # ---- constants / big persistent buffers ----
        g1_sb = constp.tile([128, NHC], FP)
        b1_sb = constp.tile([128, NHC], FP)
        g2_sb = constp.tile([128, 2], FP)
        b2_sb = constp.tile([128, 2], FP)
        nc.scalar.dma_start(g1_sb[:], g1_d[:])
        nc.scalar.dma_start(b1_sb[:], b1_d[:])
        nc.scalar.dma_start(g2_sb[:], g2_d[:])
        nc.scalar.dma_start(b2_sb[:], b2_d[:])
        ones_c = constp.tile([128, 1], BF)             # ones column (bf16 matmuls)
        nc.vector.memset(ones_c[:], 1.0)

        # q,k live in [d, row] layout for all 16 batches (chunks 0-1 q, 2-3 k)
        qkv_sb = [qkvp.tile([128, R], BF, tag=f"qkv{i}", name=f"qkv{i}")
                  for i in range(4)]

        xT_sb = [xp.tile([128, R], BF, tag=f"xT{cc}", name=f"xT{cc}")
                 for cc in range(2)]
        wq_sb = [xp.tile([128, H], BF, tag=f"wq{cc}", name=f"wq{cc}")
                 for cc in range(2)]

        # ========== Phase A.1: Gram-matrix BN1 stats -> AllReduce ==========
        # sum_r qkv[h,r]  = Wqkv @ (sum_r x_r)        (xsum = G[:,256])
        # sum_r qkv[h,r]^2 = w_h^T (x^T x) w_h = sum_c WT[c,h]*(G@WT)[c,h]
        stats = statp.tile([128, 2 * NHC], FP)
        Gsb = [statp.tile([128, C + 1], BF, tag=f"Gsb{_g}", name=f"Gsb{_g}")
               for _g in range(2)]
        # weights stream on per-engine DGE queues so the xb chunks and the
        # stats bounce DMA (sync queue) are not stuck behind 5MB of weights
        for cc in range(2):
            nc.scalar.dma_start(wq_sb[cc][:], wqkvT_d[cc * 128:(cc + 1) * 128, :])
            nc.gpsimd.dma_start(xT_sb[cc][:], xT_d[cc * 128:(cc + 1) * 128, :])
        esg = ExitStack()
        gxp = esg.enter_context(tc.tile_pool(name="gx", bufs=4))
        pgp = esg.enter_context(tc.tile_pool(name="pg", bufs=1, space="PSUM"))
        G_ps = [pgp.tile([128, C + 1], FP, tag=f"G{gg}", name=f"G{gg}")
                for gg in range(2)]
        for rq in range(10):
            # 4 row-chunks per DMA: 2KB partition lines instead of 514B
            xbt = gxp.tile([128, 4 * (C + 1)], BF, tag="xb")
            src = xb_d[rq * 512:(rq + 1) * 512, :].rearrange(
                "(ci p) c -> p ci c", ci=4)
            dst = xbt[:, :].rearrange("p (ci c) -> p ci c", ci=4)
            nc.sync.dma_start(dst, src)
            for ci in range(4):
                rc = rq * 4 + ci
                xc = xbt[:, ci * (C + 1):(ci + 1) * (C + 1)]
                for gg in range(2):
                    nc.tensor.matmul(
                        G_ps[gg][:], xc[:, gg * 128:(gg + 1) * 128],
                        xc[:], start=(rc == 0), stop=(rc == 39))
        for gg in range(2):
            nc.vector.tensor_copy(Gsb[gg][:], G_ps[gg][:])
        esg.close()
        # P1 = G @ WT ; prod = P1 .* WT ; column-sum -> sumsq row.
        # sums row from xsum (= Gsb[:,256]) @ WT. Rows land at
        # partitions 0 (sums) / 32 (sumsq) of rows_ps[nc3].
        rows_sb0 = statp.tile([1, H], BF, name="rows_sb0")  # per-h sums
        rows_sb1 = statp.tile([1, H], BF, name="rows_sb1")  # per-h sumsq
        ess = ExitStack()
        pp1 = ess.enter_context(tc.tile_pool(name="pp1", bufs=2, space="PSUM"))
        prw = ess.enter_context(tc.tile_pool(name="prw", bufs=2, space="PSUM"))
        pssp = ess.enter_context(tc.tile_pool(name="pss", bufs=1, space="PSUM"))
        scp = ess.enter_context(tc.tile_pool(name="sc", bufs=4))
        pap = ess.enter_context(tc.tile_pool(name="pa", bufs=2, space="PSUM"))

        def emit_qk_block(rb_, hc):
            pq = pap.tile([128, 512], FP, tag="pq")
            for cc in range(2):
                nc.tensor.matmul(
                    pq[:],
                    wq_sb[cc][:, hc * 128:(hc + 1) * 128],
                    xT_sb[cc][:, rb_ * 512:(rb_ + 1) * 512],
                    start=(cc == 0), stop=(cc == 1))
            if hc % 2:
                nc.scalar.copy(
                    qkv_sb[hc][:, rb_ * 512:(rb_ + 1) * 512], pq[:])
            else:
                nc.vector.tensor_copy(
                    qkv_sb[hc][:, rb_ * 512:(rb_ + 1) * 512], pq[:])

        qk_blocks = [(rb_, hc) for rb_ in range(NRB) for hc in range(4)]
        qkit = iter(qk_blocks)
        rows_ps = {}
        for nc3 in range(3):
            rows_ps[nc3] = prw.tile([128, 512], FP, tag="rows",
                                    name=f"rows{nc3}")
            for _ in range(4):
                u = next(qkit, None)
                if u is not None:
                    emit_qk_block(*u)
            for gg in range(2):
                p1 = pp1.tile([128, 512], FP, tag="p1")
                for gp in range(2):
                    nc.tensor.matmul(
                        p1[:], Gsb[gp][:, gg * 128:(gg + 1) * 128],
                        wq_sb[gp][:, nc3 * 512:(nc3 + 1) * 512],
                        start=(gp == 0), stop=(gp == 1))
                prod = scp.tile([128, 512], BF, tag="prod")
                nc.vector.tensor_tensor(
                    prod[:], p1[:],
                    wq_sb[gg][:, nc3 * 512:(nc3 + 1) * 512], OP.mult)
                nc.tensor.matmul(
                    rows_ps[nc3][32:33, :], ones_c[:, 0:1], prod[:],
                    start=(gg == 0), stop=(gg == 1),
                    tile_position=(0, 32))
            # separate accumulation group, AFTER sumsq completes
            for gg in range(2):
                nc.tensor.matmul(
                    rows_ps[nc3][0:1, :], Gsb[gg][:, 256:257],
                    wq_sb[gg][:, nc3 * 512:(nc3 + 1) * 512],
                    start=(gg == 0), stop=(gg == 1),
                    tile_position=(0, 0))
            nc.vector.tensor_copy(
                rows_sb0[0:1, nc3 * 512:(nc3 + 1) * 512],
                rows_ps[nc3][0:1, :])
            nc.vector.tensor_copy(
                rows_sb1[0:1, nc3 * 512:(nc3 + 1) * 512],
                rows_ps[nc3][32:33, :])
        # transpose rows -> per-partition stat columns [128, 24]
        stats_ps = pssp.tile([128, 2 * NHC], FP, tag="stp")
        for hc in range(NHC):
            nc.tensor.matmul(
                stats_ps[:, hc:hc + 1],
                rows_sb0[0:1, hc * 128:(hc + 1) * 128],
                ones_c[0:1, 0:1])
            nc.tensor.matmul(
                stats_ps[:, NHC + hc:NHC + hc + 1],
                rows_sb1[0:1, hc * 128:(hc + 1) * 128],
                ones_c[0:1, 0:1])
        nc.vector.tensor_copy(stats[:], stats_ps[:])
        bounce_i = dramp.tile([128, 2 * NHC], FP, tag="b1i")
        bounce_o = dramp.tile([128, 2 * NHC], FP, tag="b1o")
        nc.sync.dma_start(bounce_i[:], stats[:])
        nc.gpsimd.collective_compute(
            "AllReduce", OP.add,
            replica_groups=[list(range(NCORES))],
            ins=[bounce_i.opt()], outs=[bounce_o.opt()])

        # remaining q,k projection blocks (overlap with the collective)
        for u in qkit:
            emit_qk_block(*u)
        ess.close()

        # ---- remaining constants (overlap with collective) ----
        wprojT_sb = constp.tile([128, NH * C], BF)     # 8 d-chunks side by side
        for dc in range(NH):
            nc.gpsimd.dma_start(wprojT_sb[:, dc * C:(dc + 1) * C],
                                wprojT_d[dc * 128:(dc + 1) * 128, :])
        ebp = es.enter_context(tc.tile_pool(name="ebp", bufs=1))
        eb4_sb = []                        # [g][mc] -> [128, 4*320]
        for g in range(2):
            row = []
            for mc in range(3):
                t = ebp.tile([128, 4 * N], BF, tag=f"eb{g}_{mc}",
                             name=f"eb{g}_{mc}")
                nc.gpsimd.dma_start(t[:], eb4_d[g, mc])
                row.append(t)
            eb4_sb.append(row)


        # v row-major production helper: vb[b][g][mc] tiles [128, 512]
        # hold 4 heads' dv side by side at partitions [mb, mb+ms)
        vb_tiles = {}

        def emit_v_chunk(b, g, mc):
            ms = MCS[mc]
            mb = 64 if mc == 2 else 0
            vp_ps = pvp.tile([128, 512], FP, tag="vp")
            for cc in range(2):
                nc.tensor.matmul(
                    vp_ps[mb:mb + ms, :],
                    xT_sb[cc][:, b * N + 128 * mc:b * N + 128 * mc + ms],
                    wq_sb[cc][:, 512 + g * 512:1024 + g * 512],
                    start=(cc == 0), stop=(cc == 1),
                    tile_position=(0, mb))
            vt = vbp.tile([128, 512], BF, tag="vb")
            if g:
                nc.scalar.copy(vt[mb:mb + ms, :], vp_ps[mb:mb + ms, :])
            else:
                nc.vector.tensor_copy(vt[mb:mb + ms, :], vp_ps[mb:mb + ms, :])
            vb_tiles[(b, g, mc)] = vt

        # first two batches' v during the collective
        for b0 in range(2):
            for g in range(2):
                for mc in range(3):
                    emit_v_chunk(b0, g, mc)

        # ---- stats return + BN1 coefficients ----
        statsg = statp.tile([128, 2 * NHC], FP)
        nc.sync.dma_start(statsg[:], bounce_o[:])
        mean1 = statp.tile([128, NHC], FP)
        var1 = statp.tile([128, NHC], FP)
        tmp1 = statp.tile([128, NHC], FP)
        alpha1 = statp.tile([128, NHC], FP)
        beta1 = statp.tile([128, NHC], FP)
        nc.vector.tensor_scalar(mean1[:], statsg[:, 0:NHC], 1.0 / NT, None,
                                OP.mult)
        nc.vector.tensor_scalar(var1[:], statsg[:, NHC:2 * NHC], 1.0 / NT,
                                None, OP.mult)
        nc.vector.tensor_tensor(tmp1[:], mean1[:], mean1[:], OP.mult)
        nc.vector.tensor_tensor(var1[:], var1[:], tmp1[:], OP.subtract)
        nc.vector.tensor_scalar(var1[:], var1[:], EPS, None, OP.add)
        nc.scalar.activation(tmp1[:], var1[:], AF.Ln)
        nc.scalar.activation(var1[:], tmp1[:], AF.Exp, scale=-0.5)   # rstd
        nc.vector.tensor_tensor(alpha1[:], g1_sb[:], var1[:], OP.mult)
        nc.vector.tensor_tensor(beta1[:], mean1[:], alpha1[:], OP.mult)
        nc.vector.tensor_tensor(beta1[:], b1_sb[:], beta1[:], OP.subtract)

        def emit_qk_scale(b):
            for hc in range(4):
                eng = nc.vector if hc < 2 else nc.gpsimd
                eng.tensor_scalar(
                    qkv_sb[hc][:, b * N:(b + 1) * N],
                    qkv_sb[hc][:, b * N:(b + 1) * N],
                    alpha1[:, hc:hc + 1], beta1[:, hc:hc + 1],
                    OP.mult, OP.add)

        for b0 in range(3):
            emit_qk_scale(b0)

        # ========== Phase B+C fused batch loop ==========
        # BN2 partial sums: one column per (c2, batch)
        s2sum = statp.tile([128, 2 * BL], FP)
        s2sq = statp.tile([128, 2 * BL], FP)
        yT_sb = [qkvp.tile([128, R], BF, tag=f"yT{c2}", name=f"yT{c2}")
                 for c2 in range(2)]

        ps4p = es.enter_context(tc.tile_pool(name="ps4", bufs=2, space="PSUM"))
        pbp = es.enter_context(tc.tile_pool(name="pb", bufs=1, space="PSUM"))
        prxp = es.enter_context(tc.tile_pool(name="prx", bufs=1, space="PSUM"))
        avp = es.enter_context(tc.tile_pool(name="avg", bufs=2))
        ep = es.enter_context(tc.tile_pool(name="eb2", bufs=24))
        rrp = es.enter_context(tc.tile_pool(name="rrp", bufs=2))
        rbp = es.enter_context(tc.tile_pool(name="rbp", bufs=2))
        rcsp = es.enter_context(tc.tile_pool(name="rcs", bufs=8))
        sqp = es.enter_context(tc.tile_pool(name="sqp", bufs=2))

        proj_q = []          # pending proj jobs: (group_tile, gbase, gsz, bb, c2)

        def emit_proj_job(g_av, gbase, gsz, bb, c2):
            # yT[c2 chunk, rows of bb] = sum_h WprojT . gelu(av)
            w = gsz * N
            off = (bb - gbase) * N
            py = pbp.tile([128, N], FP, tag="py")
            for h in range(NH):
                nc.tensor.matmul(
                    py[:],
                    wprojT_sb[:, h * C + c2 * 128:h * C + c2 * 128 + 128],
                    g_av[:, h * w + off:h * w + off + N],
                    start=(h == 0), stop=(h == NH - 1))
            dst = yT_sb[c2][:, bb * N:(bb + 1) * N]
            nc.vector.tensor_scalar(
                dst, py[:], 1.0, 0.0, OP.mult, OP.add,
                accum_out=s2sum[:, c2 * BL + bb:c2 * BL + bb + 1])
            sq = sqp.tile([128, N], BF, tag="sq")
            nc.vector.scalar_tensor_tensor(
                sq[:], dst, 1.0, dst, OP.mult, OP.mult,
                accum_out=s2sq[:, c2 * BL + bb:c2 * BL + bb + 1])

        def drain_proj(k):
            for _ in range(min(k, len(proj_q))):
                emit_proj_job(*proj_q.pop(0))

        sc_tiles = {}     # (b, g) -> [mc][pair] score tiles (exp'd, biased)

        def emit_score_unit(b, g, mc):
            # QK duo into a 2-bank s4 PSUM tile per head-pair, then
            # exp+bias-mult drain it. Units for batch b+1 are woven between
            # batch b's AV work, so the PE never demand-stalls on exp.
            ms = MCS[mc]
            mb = 64 if mc == 2 else 0
            etp = []
            for pr in range(2):
                s4 = ps4p.tile([128, 1024], FP, tag="s4")
                for h2 in range(2):
                    qr = 32 * (2 * pr + h2)
                    nc.tensor.matmul(
                        s4[mb:mb + ms, h2 * 512:h2 * 512 + N],
                        qkv_sb[2 + g][qr:qr + 32,
                                      b * N + 128 * mc:
                                      b * N + 128 * mc + ms],
                        qkv_sb[g][qr:qr + 32, b * N:(b + 1) * N],
                        tile_position=(qr, mb))
                et2 = ep.tile([128, 2 * N], BF, tag="et")
                sin = s4[mb:mb + ms, :].rearrange(
                    "p (h n) -> p h n", h=2)[:, :, 0:N]
                eout = et2[mb:mb + ms, :].rearrange("p (h n) -> p h n", h=2)
                nc.scalar.activation(eout, sin, AF.Exp, scale=SCALE)
                nc.vector.tensor_tensor(
                    et2[mb:mb + ms, :], et2[mb:mb + ms, :],
                    eb4_sb[g][mc][mb:mb + ms, pr * 2 * N:(pr + 1) * 2 * N],
                    OP.mult)
                etp.append(et2)
            sc_tiles.setdefault((b, g), []).append(etp)

        def emit_rowsum(et4s):
            rp = prxp.tile([128, N], FP, tag="rx", name="rp")
            if SIM_SAFE:
                nc.vector.memset(rp[:], 1.0)
            for hh in range(4):
                rrow = 32 * hh
                for mc in range(3):
                    ms = MCS[mc]
                    mb = 64 if mc == 2 else 0
                    nc.tensor.matmul(
                        rp[rrow:rrow + 1, :],
                        ones_c[mb:mb + ms, 0:1],
                        et4s[mc][hh // 2][mb:mb + ms,
                                          (hh % 2) * N:(hh % 2 + 1) * N],
                        start=(mc == 0), stop=(mc == 2),
                        tile_position=(mb, rrow))
            rr = rrp.tile([128, N], FP, tag="rr", name="rr")
            nc.vector.reciprocal_approx_fast(rr[:], rp[:])
            # stage each head's recip row at partition 0 (the partition_broadcast
            # ucode only reads from partition 0), then broadcast across partitions
            rb = rbp.tile([128, 4 * N], BF, tag="rb")
            rr16 = rcsp.tile([1, N], BF, tag="rr16")
            nc.vector.tensor_copy(rr16[0:1, :], rr[0:1, :])
            nc.gpsimd.partition_broadcast(rb[:, 0:N], rr16[0:1, :])
            for hh in range(1, 4):
                st = rcsp.tile([1, N], BF, tag="rcs")
                nc.vector.tensor_copy(st[0:1, :], rr[32 * hh:32 * hh + 1, :])
                nc.gpsimd.partition_broadcast(
                    rb[:, hh * N:(hh + 1) * N], st[0:1, :])
            return rb

        def emit_av(b, g, et4s, rb, g_av, boff, gw, weave):
            for hh in range(4):
                h = 4 * g + hh
                av = pbp.tile([128, N], FP, tag="av")
                for mc in range(3):
                    ms = MCS[mc]
                    mb = 64 if mc == 2 else 0
                    nc.tensor.matmul(
                        av[:],
                        vb_tiles[(b, g, mc)][mb:mb + ms,
                                             hh * 128:hh * 128 + 128],
                        et4s[mc][hh // 2][mb:mb + ms,
                                          (hh % 2) * N:(hh % 2 + 1) * N],
                        start=(mc == 0), stop=(mc == 2),
                        tile_position=(mb, 0))
                nc.vector.scalar_tensor_tensor(
                    g_av[:, h * gw + boff:h * gw + boff + N],
                    av[:], 1.0, rb[:, hh * N:(hh + 1) * N],
                    OP.mult, OP.mult)
                # fillers between AV chains (hide PSUM bank reuse latency)
                if b + 2 < BL and hh in (0, 2):
                    emit_v_chunk(b + 2, g, 1 + hh // 2)
                elif hh == 1:
                    weave()

        def finalize_group(g_av, gbase, gsz, gidx=None):
            # gelu for the whole group: one contiguous op per head
            w = gsz * N
            for h in range(NH):
                gav = g_av[:, h * w:(h + 1) * w]
                nc.scalar.activation(
                    gav, gav, AF.Gelu,
                    scale=alpha1[:, 4 + h:5 + h],
                    bias=beta1[:, 4 + h:5 + h])
            for bb in range(gbase, gbase + gsz):
                for c2 in range(2):
                    proj_q.append((g_av, gbase, gsz, bb, c2))

        gi = 0            # current group index
        g_av = None
        gw = None
        pending_fin = []  # (finalize_at_batch, g_av, gbase, gsz, gidx)
        # prologue: batch 0 score ladder (overlaps phase-A tail)
        for g in range(2):
            for mc in range(3):
                emit_score_unit(0, g, mc)
        for b in range(BL):
            gbase, gsz = GROUPS[gi]
            if b == gbase:
                g_av = avp.tile([128, 2 * 8 * N], BF, tag="avg",
                                name=f"avg{gi}")
                gw = gsz * N
            while pending_fin and pending_fin[0][0] <= b:
                _, fav, fb, fs, fgi = pending_fin.pop(0)
                finalize_group(fav, fb, fs, fgi)
            drain_proj(4)
            if b + 3 < BL:
                emit_qk_scale(b + 3)
            boff = (b - gbase) * N
            # weave: batch b+1's score units between batch b's AV work
            units = [(b + 1, g2, mc2) for g2 in range(2)
                     for mc2 in range(3)] if b + 1 < BL else []
            uit = iter(units)

            def weave():
                u = next(uit, None)
                if u is not None:
                    emit_score_unit(*u)
            for g in range(2):
                et4s = sc_tiles.pop((b, g))
                rb = emit_rowsum(et4s)
                if b + 2 < BL:
                    emit_v_chunk(b + 2, g, 0)
                weave()
                emit_av(b, g, et4s, rb, g_av, boff, gw, weave)
                weave()
            for _ in range(6):
                weave()
            for g in range(2):
                for mc in range(3):
                    del vb_tiles[(b, g, mc)]
            if b == gbase + gsz - 1:
                pending_fin.append((b + (2 if b < 13 else 1), g_av, gbase,
                                    gsz, gi))
                gi += 1

        # ========== tail: drain proj, BN2, scale, store ==========
        for _, fav, fb, fs, fgi in pending_fin:
            finalize_group(fav, fb, fs, fgi)
        drain_proj(len(proj_q))
        st2 = statp.tile([128, 4], FP)
        for c2 in range(2):
            nc.vector.tensor_reduce(
                st2[:, c2:c2 + 1], s2sum[:, c2 * BL:(c2 + 1) * BL],
                mybir.AxisListType.X, OP.add)
            nc.vector.tensor_reduce(
                st2[:, 2 + c2:3 + c2], s2sq[:, c2 * BL:(c2 + 1) * BL],
                mybir.AxisListType.X, OP.add)
        b2i = dramp.tile([128, 4], FP, tag="b2i")
        b2o = dramp.tile([128, 4], FP, tag="b2o")
        nc.sync.dma_start(b2i[:], st2[:])
        nc.gpsimd.collective_compute(
            "AllReduce", OP.add,
            replica_groups=[list(range(NCORES))],
            ins=[b2i.opt()], outs=[b2o.opt()])
        st2g = statp.tile([128, 4], FP)
        nc.sync.dma_start(st2g[:], b2o[:])

        mean2 = statp.tile([128, 2], FP)
        var2 = statp.tile([128, 2], FP)
        tmp2 = statp.tile([128, 2], FP)
        alpha2 = statp.tile([128, 2], FP)
        beta2 = statp.tile([128, 2], FP)
        nc.vector.tensor_scalar(mean2[:], st2g[:, 0:2], 1.0 / NT, None, OP.mult)
        nc.vector.tensor_scalar(var2[:], st2g[:, 2:4], 1.0 / NT, None, OP.mult)
        nc.vector.tensor_tensor(tmp2[:], mean2[:], mean2[:], OP.mult)
        nc.vector.tensor_tensor(var2[:], var2[:], tmp2[:], OP.subtract)
        nc.vector.tensor_scalar(var2[:], var2[:], EPS, None, OP.add)
        nc.scalar.activation(tmp2[:], var2[:], AF.Ln)
        nc.scalar.activation(var2[:], tmp2[:], AF.Exp, scale=-0.5)
        nc.vector.tensor_tensor(alpha2[:], g2_sb[:], var2[:], OP.mult)
        nc.vector.tensor_tensor(beta2[:], mean2[:], alpha2[:], OP.mult)
        nc.vector.tensor_tensor(beta2[:], b2_sb[:], beta2[:], OP.subtract)
        if DBG:
            dstat = statp.tile([128, 48], FP)
            nc.vector.tensor_copy(dstat[:, 0:4], st2[:])
            nc.vector.tensor_copy(dstat[:, 4:8], st2g[:])
            nc.vector.tensor_copy(dstat[:, 8:10], mean2[:])
            nc.vector.tensor_copy(dstat[:, 10:12], var2[:])
            nc.vector.tensor_copy(dstat[:, 12:14], alpha2[:])
            nc.vector.tensor_copy(dstat[:, 14:16], beta2[:])
            nc.vector.tensor_copy(dstat[:, 16:48], s2sum[:])
            nc.sync.dma_start(dbg2_d[:], dstat[:])

        scp3 = es.enter_context(tc.tile_pool(name="sc3", bufs=3))
        for rb in range(NRB):
            for c2 in range(2):
                yo = scp3.tile([128, 512], BF, tag="yo")
                src = yT_sb[c2][:, rb * 512:(rb + 1) * 512]
                if (rb + c2) % 2:
                    nc.gpsimd.tensor_scalar(
                        yo[:], src, alpha2[:, c2:c2 + 1], beta2[:, c2:c2 + 1],
                        OP.mult, OP.add)
                else:
                    nc.vector.tensor_scalar(
                        yo[:], src, alpha2[:, c2:c2 + 1], beta2[:, c2:c2 + 1],
                        OP.mult, OP.add)
                qeng = (nc.sync, nc.scalar, nc.gpsimd)[(rb * 2 + c2) % 3]
                qeng.dma_start(
                    yT_d[c2 * 128:(c2 + 1) * 128, rb * 512:(rb + 1) * 512],
                    yo[:])

    nc.compile()
    return nc


_PROG = None


def _get_prog():
    global _PROG
    if _PROG is None:
        _PROG = build_program()
    return _PROG


def _host_prep(x, Wqkv, g1, b1, ab, Wproj, g2, b2, idxs):
    perm = np.empty(H, dtype=np.int64)
    for h in range(NH):
        base = h * (2 * DK + DV)
        perm[DK * h: DK * (h + 1)] = np.arange(base, base + DK)
        perm[NH * DK + DK * h: NH * DK + DK * (h + 1)] = \
            np.arange(base + DK, base + 2 * DK)
        perm[2 * NH * DK + DV * h: 2 * NH * DK + DV * (h + 1)] = \
            np.arange(base + 2 * DK, base + 2 * DK + DV)
    x = np.asarray(x, dtype=np.float32)
    Wqkv = np.asarray(Wqkv, dtype=np.float32)
    wqkvT = np.ascontiguousarray(Wqkv[perm, :].T).astype(ml_dtypes.bfloat16)
    g1c = np.ascontiguousarray(np.asarray(g1, np.float32)[perm].reshape(NHC, 128).T)
    b1c = np.ascontiguousarray(np.asarray(b1, np.float32)[perm].reshape(NHC, 128).T)
    wprojT = np.ascontiguousarray(np.asarray(Wproj, np.float32).T).astype(
        ml_dtypes.bfloat16)                                            # (1024, 256)
    E = np.exp(np.asarray(ab, np.float32))[:, np.asarray(idxs)]    # (8, 320, 320)
    eb4 = np.zeros((2, 3, 128, 4 * N), dtype=ml_dtypes.bfloat16)
    for g in range(2):
        for mc in range(3):
            ms = MCS[mc]
            mb = 64 if mc == 2 else 0
            for hh in range(4):
                eb4[g, mc, mb:mb + ms, hh * N:(hh + 1) * N] = \
                    E[4 * g + hh, 128 * mc:128 * mc + ms, :].astype(
                        ml_dtypes.bfloat16)
    common = {
        "wqkvT": wqkvT, "wprojT": wprojT, "eb4": eb4,
        "g1c": g1c, "b1c": b1c,
        "g2c": np.ascontiguousarray(
            np.asarray(g2, np.float32).reshape(2, 128).T),
        "b2c": np.ascontiguousarray(
            np.asarray(b2, np.float32).reshape(2, 128).T),
    }
    in_maps = []
    for c in range(NCORES):
        m = dict(common)
        xs = x[c * BL:(c + 1) * BL].reshape(R, C)
        m["xT"] = np.ascontiguousarray(xs.T).astype(ml_dtypes.bfloat16)
        xb = np.ones((R, C + 1), dtype=ml_dtypes.bfloat16)
        xb[:, :C] = xs.astype(ml_dtypes.bfloat16)
        m["xb"] = xb
        in_maps.append(m)
    return in_maps


def _run(in_maps, trace=False):
    nc = _get_prog()
    res = run_bass_kernel_spmd(nc, in_maps, core_ids=list(range(NCORES)),
                               trace=trace)
    out = np.concatenate(
        [np.asarray(res.results[c]["yT"]).T.reshape(BL, N, C)
         for c in range(NCORES)], axis=0)
    return out.astype(np.float32), res


def kernel(**inputs):
    out, _ = _run(_host_prep(**inputs))
    return out


def run_traced(**inputs):
    return _run(_host_prep(**inputs), trace=True)


# revision 43
# speedup vs baseline: 1.1012x; 1.1012x over previous
"""Trainium2 Bass kernel for nn_Attention (qkv+BN -> biased softmax attention -> gelu -> proj+BN).

Sharding: data-parallel over batch B=128 across 8 NeuronCores (16 batches each).
BatchNorm (training-mode) statistics are all-reduced across cores (tiny collectives).

v2 layout strategy (vs v1): keep the PE tensor engine continuously busy so the
TRN2 p-state ramp holds at full clock, and strip non-essential PE work.
  - Phase A: Gram-matrix BN1 stats computed FIRST; the stats AllReduce overlaps
    the q,k projection matmuls and the first two batches' v-production.
  - q,k BN scaling applied lazily per batch (spread across the loop).
  - softmax divide: reciprocal row-sums broadcast across partitions on GpSimd
    (partition_broadcast) instead of PE ones-outer matmuls; the divide itself is
    fused into the AV PSUM->SBUF copy via scalar_tensor_tensor.
  - attention outputs collect into per-4-batch av_group tiles [128, 4*8*320];
    gelu runs once per head per group (strided AP), so Exp<->Gelu activation
    table switches amortize over 4 batches.
  - output projection (proj) chains for group G are drained as PE filler during
    group G+1's batches; BN2 sums/sumsq accumulate per chunk via accum_out.
  - v row-major production for batch b+2 is interleaved between AV chains as PE
    filler (also hides PSUM bank reuse latency).
"""
import os
from contextlib import ExitStack

import numpy as np
import ml_dtypes

import concourse.bass as bass
import concourse.tile as tile
from concourse import bacc, mybir
from concourse.bass_utils import run_bass_kernel_spmd

NCORES = int(os.environ.get("KERN_NCORES", "8"))
TRACE_SIM = os.environ.get("KERN_TRACE_SIM", "") == "1"
DBG = os.environ.get("KERN_DBG", "") == "1"
SIM_SAFE = os.environ.get("KERN_SIM_SAFE", "") == "1"
B, N, C = 128, 320, 256
NH, DK, DV = 8, 32, 128
H = NH * (2 * DK + DV)       # 1536
DH = NH * DV                 # 1024
BL = B // 8                  # 16 batches/core (fixed shard size)
R = BL * N                   # 5120 rows/core
NT = B * N                   # 40960 global rows
EPS = 1e-5
SCALE = DK ** -0.5
FP = mybir.dt.float32
BF = mybir.dt.bfloat16

NHC = H // 128               # 12 h-chunks
NRB = R // 512               # 10 row blocks of 512
MCS = [128, 128, 64]         # chunking of N=320
AF = mybir.ActivationFunctionType
OP = mybir.AluOpType

# gelu/proj group boundaries (batch index ranges)
GROUPS = [(2 * i, 2) for i in range(7)] + [(14, 1), (15, 1)]


def build_program():
    nc = bacc.Bacc("TRN2", target_bir_lowering=False, debug=False,
                   enable_asserts=False, num_devices=NCORES)
    xT_d = nc.dram_tensor("xT", [C, R], BF, kind="ExternalInput").ap()
    xb_d = nc.dram_tensor("xb", [R, C + 1], BF, kind="ExternalInput").ap()
    wqkvT_d = nc.dram_tensor("wqkvT", [C, H], BF, kind="ExternalInput").ap()
    wprojT_d = nc.dram_tensor("wprojT", [DH, C], BF, kind="ExternalInput").ap()
    eb4_d = nc.dram_tensor("eb4", [2, 3, 128, 4 * N], BF, kind="ExternalInput").ap()
    g1_d = nc.dram_tensor("g1c", [128, NHC], FP, kind="ExternalInput").ap()
    b1_d = nc.dram_tensor("b1c", [128, NHC], FP, kind="ExternalInput").ap()
    g2_d = nc.dram_tensor("g2c", [128, 2], FP, kind="ExternalInput").ap()
    b2_d = nc.dram_tensor("b2c", [128, 2], FP, kind="ExternalInput").ap()
    yT_d = nc.dram_tensor("yT", [C, R], BF, kind="ExternalOutput").ap()
    dbg_d = nc.dram_tensor("dbg", [128, 5120], BF,
                           kind="ExternalOutput").ap() if DBG else None
    dbg2_d = nc.dram_tensor("dbg2", [128, 48], FP,
                            kind="ExternalOutput").ap() if DBG else None

    es = ExitStack()
    with es:
        tc = es.enter_context(tile.TileContext(nc, trace_sim=TRACE_SIM))
        constp = es.enter_context(tc.tile_pool(name="const", bufs=1))
        qkvp = es.enter_context(tc.tile_pool(name="qkv", bufs=1))
        dramp = es.enter_context(tc.tile_pool(name="dram", bufs=1, space="DRAM"))
        statp = es.enter_context(tc.tile_pool(name="stat", bufs=1))
        xp = es.enter_context(tc.tile_pool(name="xa", bufs=1))
        pvp = es.enter_context(tc.tile_pool(name="pv", bufs=1, space="PSUM"))
        vbp = es.enter_context(tc.tile_pool(name="vb", bufs=18))

        # ---- constants / big persistent buffers ----
        g1_sb = constp.tile([128, NHC], FP)
        b1_sb = constp.tile([128, NHC], FP)
        g2_sb = constp.tile([128, 2], FP)
        b2_sb = constp.tile([128, 2], FP)
        nc.scalar.dma_start(g1_sb[:], g1_d[:])
        nc.scalar.dma_start(b1_sb[:], b1_d[:])
        nc.scalar.dma_start(g2_sb[:], g2_d[:])
        nc.scalar.dma_start(b2_sb[:], b2_d[:])
        ones_c = constp.tile([128, 1], BF)             # ones column (bf16 matmuls)
        nc.vector.memset(ones_c[:], 1.0)

        # q,k live in [d, row] layout for all 16 batches (chunks 0-1 q, 2-3 k)
        qkv_sb = [qkvp.tile([128, R], BF, tag=f"qkv{i}", name=f"qkv{i}")
                  for i in range(4)]

        xT_sb = [xp.tile([128, R], BF, tag=f"xT{cc}", name=f"xT{cc}")
                 for cc in range(2)]
        wq_sb = [xp.tile([128, H], BF, tag=f"wq{cc}", name=f"wq{cc}")
                 for cc in range(2)]

        # ========== Phase A.1: Gram-matrix BN1 stats -> AllReduce ==========
        # sum_r qkv[h,r]  = Wqkv @ (sum_r x_r)        (xsum = G[:,256])
        # sum_r qkv[h,r]^2 = w_h^T (x^T x) w_h = sum_c WT[c,h]*(G@WT)[c,h]
        stats = statp.tile([128, 2 * NHC], FP)
        Gsb = [statp.tile([128, C + 1], BF, tag=f"Gsb{_g}", name=f"Gsb{_g}")
               for _g in range(2)]
        # weights stream on per-engine DGE queues so the xb chunks and the
        # stats bounce DMA (sync queue) are not stuck behind 5MB of weights
        for cc in range(2):
            nc.scalar.dma_start(wq_sb[cc][:], wqkvT_d[cc * 128:(cc + 1) * 128, :])
            nc.gpsimd.dma_start(xT_sb[cc][:], xT_d[cc * 128:(cc + 1) * 128, :])
        esg = ExitStack()
        gxp = esg.enter_context(tc.tile_pool(name="gx", bufs=4))
        pgp = esg.enter_context(tc.tile_pool(name="pg", bufs=1, space="PSUM"))
        G_ps = [pgp.tile([128, C + 1], FP, tag=f"G{gg}", name=f"G{gg}")
                for gg in range(2)]
        for rq in range(10):
            # 4 row-chunks per DMA: 2KB partition lines instead of 514B
            xbt = gxp.tile([128, 4 * (C + 1)], BF, tag="xb")
            src = xb_d[rq * 512:(rq + 1) * 512, :].rearrange(
                "(ci p) c -> p ci c", ci=4)
            dst = xbt[:, :].rearrange("p (ci c) -> p ci c", ci=4)
            nc.sync.dma_start(dst, src)
            for ci in range(4):
                rc = rq * 4 + ci
                xc = xbt[:, ci * (C + 1):(ci + 1) * (C + 1)]
                for gg in range(2):
                    nc.tensor.matmul(
                        G_ps[gg][:], xc[:, gg * 128:(gg + 1) * 128],
                        xc[:], start=(rc == 0), stop=(rc == 39))
        for gg in range(2):
            nc.vector.tensor_copy(Gsb[gg][:], G_ps[gg][:])
        esg.close()
        # P1 = G @ WT ; prod = P1 .* WT ; column-sum -> sumsq row.
        # sums row from xsum (= Gsb[:,256]) @ WT. Rows land at
        # partitions 0 (sums) / 32 (sumsq) of rows_ps[nc3].
        rows_sb0 = statp.tile([1, H], BF, name="rows_sb0")  # per-h sums
        rows_sb1 = statp.tile([1, H], BF, name="rows_sb1")  # per-h sumsq
        ess = ExitStack()
        pp1 = ess.enter_context(tc.tile_pool(name="pp1", bufs=2, space="PSUM"))
        prw = ess.enter_context(tc.tile_pool(name="prw", bufs=2, space="PSUM"))
        pssp = ess.enter_context(tc.tile_pool(name="pss", bufs=1, space="PSUM"))
        scp = ess.enter_context(tc.tile_pool(name="sc", bufs=4))
        pap = ess.enter_context(tc.tile_pool(name="pa", bufs=2, space="PSUM"))

        def emit_qk_block(rb_, hc):
            pq = pap.tile([128, 512], FP, tag="pq")
            for cc in range(2):
                nc.tensor.matmul(
                    pq[:],
                    wq_sb[cc][:, hc * 128:(hc + 1) * 128],
                    xT_sb[cc][:, rb_ * 512:(rb_ + 1) * 512],
                    start=(cc == 0), stop=(cc == 1))
            if hc % 2:
                nc.scalar.copy(
                    qkv_sb[hc][:, rb_ * 512:(rb_ + 1) * 512], pq[:])
            else:
                nc.vector.tensor_copy(
                    qkv_sb[hc][:, rb_ * 512:(rb_ + 1) * 512], pq[:])

        qk_blocks = [(rb_, hc) for rb_ in range(NRB) for hc in range(4)]
        qkit = iter(qk_blocks)
        rows_ps = {}
        for nc3 in range(3):
            rows_ps[nc3] = prw.tile([128, 512], FP, tag="rows",
                                    name=f"rows{nc3}")
            for _ in range(4):
                u = next(qkit, None)
                if u is not None:
                    emit_qk_block(*u)
            for gg in range(2):
                p1 = pp1.tile([128, 512], FP, tag="p1")
                for gp in range(2):
                    nc.tensor.matmul(
                        p1[:], Gsb[gp][:, gg * 128:(gg + 1) * 128],
                        wq_sb[gp][:, nc3 * 512:(nc3 + 1) * 512],
                        start=(gp == 0), stop=(gp == 1))
                prod = scp.tile([128, 512], BF, tag="prod")
                nc.vector.tensor_tensor(
                    prod[:], p1[:],
                    wq_sb[gg][:, nc3 * 512:(nc3 + 1) * 512], OP.mult)
                nc.tensor.matmul(
                    rows_ps[nc3][32:33, :], ones_c[:, 0:1], prod[:],
                    start=(gg == 0), stop=(gg == 1),
                    tile_position=(0, 32))
            # separate accumulation group, AFTER sumsq completes
            for gg in range(2):
                nc.tensor.matmul(
                    rows_ps[nc3][0:1, :], Gsb[gg][:, 256:257],
                    wq_sb[gg][:, nc3 * 512:(nc3 + 1) * 512],
                    start=(gg == 0), stop=(gg == 1),
                    tile_position=(0, 0))
            nc.vector.tensor_copy(
                rows_sb0[0:1, nc3 * 512:(nc3 + 1) * 512],
                rows_ps[nc3][0:1, :])
            nc.vector.tensor_copy(
                rows_sb1[0:1, nc3 * 512:(nc3 + 1) * 512],
                rows_ps[nc3][32:33, :])
        # transpose rows -> per-partition stat columns [128, 24]
        stats_ps = pssp.tile([128, 2 * NHC], FP, tag="stp")
        for hc in range(NHC):
            nc.tensor.matmul(
                stats_ps[:, hc:hc + 1],
                rows_sb0[0:1, hc * 128:(hc + 1) * 128],
                ones_c[0:1, 0:1])
            nc.tensor.matmul(
                stats_ps[:, NHC + hc:NHC + hc + 1],
                rows_sb1[0:1, hc * 128:(hc + 1) * 128],
                ones_c[0:1, 0:1])
        nc.vector.tensor_copy(stats[:], stats_ps[:])
        bounce_i = dramp.tile([128, 2 * NHC], FP, tag="b1i")
        bounce_o = dramp.tile([128, 2 * NHC], FP, tag="b1o")
        nc.sync.dma_start(bounce_i[:], stats[:])
        nc.gpsimd.collective_compute(
            "AllReduce", OP.add,
            replica_groups=[list(range(NCORES))],
            ins=[bounce_i.opt()], outs=[bounce_o.opt()])

        # remaining q,k projection blocks (overlap with the collective)
        for u in qkit:
            emit_qk_block(*u)
        ess.close()

        # ---- remaining constants (overlap with collective) ----
        wprojT_sb = constp.tile([128, NH * C], BF)     # 8 d-chunks side by side
        for dc in range(NH):
            nc.gpsimd.dma_start(wprojT_sb[:, dc * C:(dc + 1) * C],
                                wprojT_d[dc * 128:(dc + 1) * 128, :])
        ebp = es.enter_context(tc.tile_pool(name="ebp", bufs=1))
        eb4_sb = []                        # [g][mc] -> [128, 4*320]
        for g in range(2):
            row = []
            for mc in range(3):
                t = ebp.tile([128, 4 * N], BF, tag=f"eb{g}_{mc}",
                             name=f"eb{g}_{mc}")
                nc.gpsimd.dma_start(t[:], eb4_d[g, mc])
                row.append(t)
            eb4_sb.append(row)


        # v row-major production helper: vb[b][g][mc] tiles [128, 512]
        # hold 4 heads' dv side by side at partitions [mb, mb+ms)
        vb_tiles = {}

        def emit_v_chunk(b, g, mc):
            ms = MCS[mc]
            mb = 64 if mc == 2 else 0
            vp_ps = pvp.tile([128, 512], FP, tag="vp")
            for cc in range(2):
                nc.tensor.matmul(
                    vp_ps[mb:mb + ms, :],
                    xT_sb[cc][:, b * N + 128 * mc:b * N + 128 * mc + ms],
                    wq_sb[cc][:, 512 + g * 512:1024 + g * 512],
                    start=(cc == 0), stop=(cc == 1),
                    tile_position=(0, mb))
            vt = vbp.tile([128, 512], BF, tag="vb")
            if g:
                nc.scalar.copy(vt[mb:mb + ms, :], vp_ps[mb:mb + ms, :])
            else:
                nc.vector.tensor_copy(vt[mb:mb + ms, :], vp_ps[mb:mb + ms, :])
            vb_tiles[(b, g, mc)] = vt

        # first two batches' v during the collective
        for b0 in range(2):
            for g in range(2):
                for mc in range(3):
                    emit_v_chunk(b0, g, mc)

        # ---- stats return + BN1 coefficients ----
        statsg = statp.tile([128, 2 * NHC], FP)
        nc.sync.dma_start(statsg[:], bounce_o[:])
        mean1 = statp.tile([128, NHC], FP)
        var1 = statp.tile([128, NHC], FP)
        tmp1 = statp.tile([128, NHC], FP)
        alpha1 = statp.tile([128, NHC], FP)
        beta1 = statp.tile([128, NHC], FP)
        nc.vector.tensor_scalar(mean1[:], statsg[:, 0:NHC], 1.0 / NT, None,
                                OP.mult)
        nc.vector.tensor_scalar(var1[:], statsg[:, NHC:2 * NHC], 1.0 / NT,
                                None, OP.mult)
        nc.vector.tensor_tensor(tmp1[:], mean1[:], mean1[:], OP.mult)
        nc.vector.tensor_tensor(var1[:], var1[:], tmp1[:], OP.subtract)
        nc.vector.tensor_scalar(var1[:], var1[:], EPS, None, OP.add)
        nc.scalar.activation(tmp1[:], var1[:], AF.Ln)
        nc.scalar.activation(var1[:], tmp1[:], AF.Exp, scale=-0.5)   # rstd
        nc.vector.tensor_tensor(alpha1[:], g1_sb[:], var1[:], OP.mult)
        nc.vector.tensor_tensor(beta1[:], mean1[:], alpha1[:], OP.mult)
        nc.vector.tensor_tensor(beta1[:], b1_sb[:], beta1[:], OP.subtract)

        def emit_qk_scale(b):
            for hc in range(4):
                eng = nc.vector if hc < 2 else nc.gpsimd
                eng.tensor_scalar(
                    qkv_sb[hc][:, b * N:(b + 1) * N],
                    qkv_sb[hc][:, b * N:(b + 1) * N],
                    alpha1[:, hc:hc + 1], beta1[:, hc:hc + 1],
                    OP.mult, OP.add)

        for b0 in range(3):
            emit_qk_scale(b0)

        # ========== Phase B+C fused batch loop ==========
        # BN2 partial sums: one column per (c2, batch)
        s2sum = statp.tile([128, 2 * BL], FP)
        s2sq = statp.tile([128, 2 * BL], FP)
        yT_sb = [qkvp.tile([128, R], BF, tag=f"yT{c2}", name=f"yT{c2}")
                 for c2 in range(2)]

        ps4p = es.enter_context(tc.tile_pool(name="ps4", bufs=2, space="PSUM"))
        pbp = es.enter_context(tc.tile_pool(name="pb", bufs=1, space="PSUM"))
        prxp = es.enter_context(tc.tile_pool(name="prx", bufs=1, space="PSUM"))
        avp = es.enter_context(tc.tile_pool(name="avg", bufs=2))
        ep = es.enter_context(tc.tile_pool(name="eb2", bufs=24))
        rrp = es.enter_context(tc.tile_pool(name="rrp", bufs=2))
        rbp = es.enter_context(tc.tile_pool(name="rbp", bufs=2))
        rcsp = es.enter_context(tc.tile_pool(name="rcs", bufs=8))
        sqp = es.enter_context(tc.tile_pool(name="sqp", bufs=2))

        proj_q = []          # pending proj jobs: (group_tile, gbase, gsz, bb, c2)

        def emit_proj_job(g_av, gbase, gsz, bb, c2):
            # yT[c2 chunk, rows of bb] = sum_h WprojT . gelu(av)
            w = gsz * N
            off = (bb - gbase) * N
            py = pbp.tile([128, N], FP, tag="py")
            for h in range(NH):
                nc.tensor.matmul(
                    py[:],
                    wprojT_sb[:, h * C + c2 * 128:h * C + c2 * 128 + 128],
                    g_av[:, h * w + off:h * w + off + N],
                    start=(h == 0), stop=(h == NH - 1))
            dst = yT_sb[c2][:, bb * N:(bb + 1) * N]
            nc.vector.tensor_scalar(
                dst, py[:], 1.0, 0.0, OP.mult, OP.add,
                accum_out=s2sum[:, c2 * BL + bb:c2 * BL + bb + 1])
            sq = sqp.tile([128, N], BF, tag="sq")
            nc.vector.scalar_tensor_tensor(
                sq[:], dst, 1.0, dst, OP.mult, OP.mult,
                accum_out=s2sq[:, c2 * BL + bb:c2 * BL + bb + 1])

        def drain_proj(k):
            for _ in range(min(k, len(proj_q))):
                emit_proj_job(*proj_q.pop(0))

        sc_tiles = {}     # (b, g) -> [mc][pair] score tiles (exp'd, biased)

        def emit_score_unit(b, g, mc):
            # QK duo into a 2-bank s4 PSUM tile per head-pair, then
            # exp+bias-mult drain it. Units for batch b+1 are woven between
            # batch b's AV work, so the PE never demand-stalls on exp.
            ms = MCS[mc]
            mb = 64 if mc == 2 else 0
            etp = []
            for pr in range(2):
                s4 = ps4p.tile([128, 1024], FP, tag="s4")
                for h2 in range(2):
                    qr = 32 * (2 * pr + h2)
                    nc.tensor.matmul(
                        s4[mb:mb + ms, h2 * 512:h2 * 512 + N],
                        qkv_sb[2 + g][qr:qr + 32,
                                      b * N + 128 * mc:
                                      b * N + 128 * mc + ms],
                        qkv_sb[g][qr:qr + 32, b * N:(b + 1) * N],
                        tile_position=(qr, mb))
                et2 = ep.tile([128, 2 * N], BF, tag="et")
                sin = s4[mb:mb + ms, :].rearrange(
                    "p (h n) -> p h n", h=2)[:, :, 0:N]
                eout = et2[mb:mb + ms, :].rearrange("p (h n) -> p h n", h=2)
                nc.scalar.activation(eout, sin, AF.Exp, scale=SCALE)
                nc.vector.tensor_tensor(
                    et2[mb:mb + ms, :], et2[mb:mb + ms, :],
                    eb4_sb[g][mc][mb:mb + ms, pr * 2 * N:(pr + 1) * 2 * N],
                    OP.mult)
                etp.append(et2)
            sc_tiles.setdefault((b, g), []).append(etp)

        def emit_rowsum(et4s):
            rp = prxp.tile([128, N], FP, tag="rx", name="rp")
            if SIM_SAFE:
                nc.vector.memset(rp[:], 1.0)
            for hh in range(4):
                rrow = 32 * hh
                for mc in range(3):
                    ms = MCS[mc]
                    mb = 64 if mc == 2 else 0
                    nc.tensor.matmul(
                        rp[rrow:rrow + 1, :],
                        ones_c[mb:mb + ms, 0:1],
                        et4s[mc][hh // 2][mb:mb + ms,
                                          (hh % 2) * N:(hh % 2 + 1) * N],
                        start=(mc == 0), stop=(mc == 2),
                        tile_position=(mb, rrow))
            rr = rrp.tile([128, N], FP, tag="rr", name="rr")
            nc.vector.reciprocal_approx_fast(rr[:], rp[:])
            # stage each head's recip row at partition 0 (the partition_broadcast
            # ucode only reads from partition 0), then broadcast across partitions
            rb = rbp.tile([128, 4 * N], BF, tag="rb")
            rr16 = rcsp.tile([1, N], BF, tag="rr16")
            nc.vector.tensor_copy(rr16[0:1, :], rr[0:1, :])
            nc.gpsimd.partition_broadcast(rb[:, 0:N], rr16[0:1, :])
            for hh in range(1, 4):
                st = rcsp.tile([1, N], BF, tag="rcs")
                nc.vector.tensor_copy(st[0:1, :], rr[32 * hh:32 * hh + 1, :])
                nc.gpsimd.partition_broadcast(
                    rb[:, hh * N:(hh + 1) * N], st[0:1, :])
            return rb

        def emit_av(b, g, et4s, rb, g_av, boff, gw, weave):
            for hh in range(4):
                h = 4 * g + hh
                av = pbp.tile([128, N], FP, tag="av")
                for mc in range(3):
                    ms = MCS[mc]
                    mb = 64 if mc == 2 else 0
                    nc.tensor.matmul(
                        av[:],
                        vb_tiles[(b, g, mc)][mb:mb + ms,
                                             hh * 128:hh * 128 + 128],
                        et4s[mc][hh // 2][mb:mb + ms,
                                          (hh % 2) * N:(hh % 2 + 1) * N],
                        start=(mc == 0), stop=(mc == 2),
                        tile_position=(mb, 0))
                nc.vector.scalar_tensor_tensor(
                    g_av[:, h * gw + boff:h * gw + boff + N],
                    av[:], 1.0, rb[:, hh * N:(hh + 1) * N],
                    OP.mult, OP.mult)
                # fillers between AV chains (hide PSUM bank reuse latency)
                if b + 2 < BL and hh in (0, 2):
                    emit_v_chunk(b + 2, g, 1 + hh // 2)
                elif hh == 1:
                    weave()

        def finalize_group(g_av, gbase, gsz, gidx=None):
            # gelu for the whole group: one contiguous op per head
            w = gsz * N
            for h in range(NH):
                gav = g_av[:, h * w:(h + 1) * w]
                nc.scalar.activation(
                    gav, gav, AF.Gelu,
                    scale=alpha1[:, 4 + h:5 + h],
                    bias=beta1[:, 4 + h:5 + h])
            for bb in range(gbase, gbase + gsz):
                for c2 in range(2):
                    proj_q.append((g_av, gbase, gsz, bb, c2))

        gi = 0            # current group index
        g_av = None
        gw = None
        pending_fin = []  # (finalize_at_batch, g_av, gbase, gsz, gidx)
        # prologue: batch 0 score ladder (overlaps phase-A tail)
        for g in range(2):
            for mc in range(3):
                emit_score_unit(0, g, mc)
        for b in range(BL):
            gbase, gsz = GROUPS[gi]
            if b == gbase:
                g_av = avp.tile([128, 2 * 8 * N], BF, tag="avg",
                                name=f"avg{gi}")
                gw = gsz * N
            while pending_fin and pending_fin[0][0] <= b:
                _, fav, fb, fs, fgi = pending_fin.pop(0)
                finalize_group(fav, fb, fs, fgi)
            drain_proj(2)
            if b + 3 < BL:
                emit_qk_scale(b + 3)
            boff = (b - gbase) * N
            # weave: batch b+1's score units between batch b's AV work
            units = [(b + 1, g2, mc2) for g2 in range(2)
                     for mc2 in range(3)] if b + 1 < BL else []
            uit = iter(units)

            def weave():
                u = next(uit, None)
                if u is not None:
                    emit_score_unit(*u)
            for g in range(2):
                et4s = sc_tiles.pop((b, g))
                rb = emit_rowsum(et4s)
                if b + 2 < BL:
                    emit_v_chunk(b + 2, g, 0)
                weave()
                emit_av(b, g, et4s, rb, g_av, boff, gw, weave)
                weave()
            for _ in range(6):
                weave()
            for g in range(2):
                for mc in range(3):
                    del vb_tiles[(b, g, mc)]
            if b == gbase + gsz - 1:
                pending_fin.append((b + (2 if b < 13 else 1), g_av, gbase,
                                    gsz, gi))
                gi += 1

        # ========== tail: drain proj, BN2, scale, store ==========
        for _, fav, fb, fs, fgi in pending_fin:
            finalize_group(fav, fb, fs, fgi)
        drain_proj(len(proj_q))
        st2 = statp.tile([128, 4], FP)
        for c2 in range(2):
            nc.vector.tensor_reduce(
                st2[:, c2:c2 + 1], s2sum[:, c2 * BL:(c2 + 1) * BL],
                mybir.AxisListType.X, OP.add)
            nc.vector.tensor_reduce(
                st2[:, 2 + c2:3 + c2], s2sq[:, c2 * BL:(c2 + 1) * BL],
                mybir.AxisListType.X, OP.add)
        b2i = dramp.tile([128, 4], FP, tag="b2i")
        b2o = dramp.tile([128, 4], FP, tag="b2o")
        nc.sync.dma_start(b2i[:], st2[:])
        nc.gpsimd.collective_compute(
            "AllReduce", OP.add,
            replica_groups=[list(range(NCORES))],
            ins=[b2i.opt()], outs=[b2o.opt()])
        st2g = statp.tile([128, 4], FP)
        nc.sync.dma_start(st2g[:], b2o[:])

        mean2 = statp.tile([128, 2], FP)
        var2 = statp.tile([128, 2], FP)
        tmp2 = statp.tile([128, 2], FP)
        alpha2 = statp.tile([128, 2], FP)
        beta2 = statp.tile([128, 2], FP)
        nc.vector.tensor_scalar(mean2[:], st2g[:, 0:2], 1.0 / NT, None, OP.mult)
        nc.vector.tensor_scalar(var2[:], st2g[:, 2:4], 1.0 / NT, None, OP.mult)
        nc.vector.tensor_tensor(tmp2[:], mean2[:], mean2[:], OP.mult)
        nc.vector.tensor_tensor(var2[:], var2[:], tmp2[:], OP.subtract)
        nc.vector.tensor_scalar(var2[:], var2[:], EPS, None, OP.add)
        nc.scalar.activation(tmp2[:], var2[:], AF.Ln)
        nc.scalar.activation(var2[:], tmp2[:], AF.Exp, scale=-0.5)
        nc.vector.tensor_tensor(alpha2[:], g2_sb[:], var2[:], OP.mult)
        nc.vector.tensor_tensor(beta2[:], mean2[:], alpha2[:], OP.mult)
        nc.vector.tensor_tensor(beta2[:], b2_sb[:], beta2[:], OP.subtract)
        if DBG:
            dstat = statp.tile([128, 48], FP)
            nc.vector.tensor_copy(dstat[:, 0:4], st2[:])
            nc.vector.tensor_copy(dstat[:, 4:8], st2g[:])
            nc.vector.tensor_copy(dstat[:, 8:10], mean2[:])
            nc.vector.tensor_copy(dstat[:, 10:12], var2[:])
            nc.vector.tensor_copy(dstat[:, 12:14], alpha2[:])
            nc.vector.tensor_copy(dstat[:, 14:16], beta2[:])
            nc.vector.tensor_copy(dstat[:, 16:48], s2sum[:])
            nc.sync.dma_start(dbg2_d[:], dstat[:])

        scp3 = es.enter_context(tc.tile_pool(name="sc3", bufs=3))
        for rb in range(NRB):
            for c2 in range(2):
                yo = scp3.tile([128, 512], BF, tag="yo")
                src = yT_sb[c2][:, rb * 512:(rb + 1) * 512]
                if (rb + c2) % 2:
                    nc.gpsimd.tensor_scalar(
                        yo[:], src, alpha2[:, c2:c2 + 1], beta2[:, c2:c2 + 1],
                        OP.mult, OP.add)
                else:
                    nc.vector.tensor_scalar(
                        yo[:], src, alpha2[:, c2:c2 + 1], beta2[:, c2:c2 + 1],
                        OP.mult, OP.add)
                qeng = (nc.sync, nc.scalar, nc.gpsimd)[(rb * 2 + c2) % 3]
                qeng.dma_start(
                    yT_d[c2 * 128:(c2 + 1) * 128, rb * 512:(rb + 1) * 512],
                    yo[:])

    nc.compile()
    return nc


_PROG = None


def _get_prog():
    global _PROG
    if _PROG is None:
        _PROG = build_program()
    return _PROG


def _host_prep(x, Wqkv, g1, b1, ab, Wproj, g2, b2, idxs):
    perm = np.empty(H, dtype=np.int64)
    for h in range(NH):
        base = h * (2 * DK + DV)
        perm[DK * h: DK * (h + 1)] = np.arange(base, base + DK)
        perm[NH * DK + DK * h: NH * DK + DK * (h + 1)] = \
            np.arange(base + DK, base + 2 * DK)
        perm[2 * NH * DK + DV * h: 2 * NH * DK + DV * (h + 1)] = \
            np.arange(base + 2 * DK, base + 2 * DK + DV)
    x = np.asarray(x, dtype=np.float32)
    Wqkv = np.asarray(Wqkv, dtype=np.float32)
    wqkvT = np.ascontiguousarray(Wqkv[perm, :].T).astype(ml_dtypes.bfloat16)
    g1c = np.ascontiguousarray(np.asarray(g1, np.float32)[perm].reshape(NHC, 128).T)
    b1c = np.ascontiguousarray(np.asarray(b1, np.float32)[perm].reshape(NHC, 128).T)
    wprojT = np.ascontiguousarray(np.asarray(Wproj, np.float32).T).astype(
        ml_dtypes.bfloat16)                                            # (1024, 256)
    E = np.exp(np.asarray(ab, np.float32))[:, np.asarray(idxs)]    # (8, 320, 320)
    eb4 = np.zeros((2, 3, 128, 4 * N), dtype=ml_dtypes.bfloat16)
    for g in range(2):
        for mc in range(3):
            ms = MCS[mc]
            mb = 64 if mc == 2 else 0
            for hh in range(4):
                eb4[g, mc, mb:mb + ms, hh * N:(hh + 1) * N] = \
                    E[4 * g + hh, 128 * mc:128 * mc + ms, :].astype(
                        ml_dtypes.bfloat16)
    common = {
        "wqkvT": wqkvT, "wprojT": wprojT, "eb4": eb4,
        "g1c": g1c, "b1c": b1c,
        "g2c": np.ascontiguousarray(
            np.asarray(g2, np.float32).reshape(2, 128).T),
        "b2c": np.ascontiguousarray(
            np.asarray(b2, np.float32).reshape(2, 128).T),
    }
    in_maps = []
    for c in range(NCORES):
        m = dict(common)
        xs = x[c * BL:(c + 1) * BL].reshape(R, C)
        m["xT"] = np.ascontiguousarray(xs.T).astype(ml_dtypes.bfloat16)
        xb = np.ones((R, C + 1), dtype=ml_dtypes.bfloat16)
        xb[:, :C] = xs.astype(ml_dtypes.bfloat16)
        m["xb"] = xb
        in_maps.append(m)
    return in_maps


def _run(in_maps, trace=False):
    nc = _get_prog()
    res = run_bass_kernel_spmd(nc, in_maps, core_ids=list(range(NCORES)),
                               trace=trace)
    out = np.concatenate(
        [np.asarray(res.results[c]["yT"]).T.reshape(BL, N, C)
         for c in range(NCORES)], axis=0)
    return out.astype(np.float32), res


def kernel(**inputs):
    out, _ = _run(_host_prep(**inputs))
    return out


def run_traced(**inputs):
    return _run(_host_prep(**inputs), trace=True)
